# revision 5
# baseline (speedup 1.0000x reference)
"""ARD-RBF covariance kernel for Trainium2 (Bass/Tile), 8-core row-parallel.

Math (matches the reference):
    s  = exp(-weights[:, 0])                      # (D,) inverse lengthscales
    Us = U * s ; Vs = V * s
    sq[i, j] = ||Us_i||^2 + ||Vs_j||^2 - 2 Us_i . Vs_j
    K[i, j]  = exp(2*sn) * exp(-0.5 * max(sq, 0))

Device strategy (per core, rows sharded 8 ways):
    The augmented operands are built on the HOST (U/V are only 8192x16, so
    prep is trivial numpy) and DMA'd in ready-to-use:
      L (114 x rows): 4 replicas, at partitions 0/32/64/96, of
          [-2*Us^T block ; ||Us||^2 row ; ones row]   (K = 18)
      R (114 x m_cols): 4 replicas of [Vs^T ; ones row ; ||Vs||^2 row]
    One augmented matmul then computes sq directly in PSUM; the 4 replicas
    let 4 matmuls run concurrently in the PE's four 32-row groups
    (tile_position row tiling), hiding fp32 LDWEIGHTS + 2-pass cost.
    A single ScalarE activation per [128, 2048] PSUM tile computes
      out = Exp(-0.5 * psum + 2*sn)   (PSUM -> SBUF, bf16 output)
    and the bf16 tiles are DMA'd out in 1 MB chunks; the host upcasts to
    fp32 (bf16 rounding is ~2e-3 relative, inside the 2e-2 gate).

Per core the bf16 output is 16 MB (~45 us at 358 GB/s HBM write); the
8.4M-element Exp on ScalarE (~1 elem/cycle/lane @ 1.2 GHz => ~60 us) is
the steady-state bottleneck, with PE and DMA hidden under it.
"""

import numpy as np

import concourse.bacc as bacc
import concourse.bass as bass  # noqa: F401  (AP helpers)
import concourse.mybir as mybir
import concourse.tile as tile

N, M, D = 8192, 8192, 16
N_CORES = 8
ROWS = N // N_CORES  # 1024 rows of U per core
P = 128              # output partitions per row block
FREE = 512           # matmul moving free dim (fp32 max)
QUAD = 2048          # one ACT instruction: 4 PSUM banks of f32
HALF = 4096          # one output store: [128, 4096] fp16 = 1 MB
K = D + 2            # augmented contraction dim
GAP = 32             # partition stride between the 4 operand replicas
AUG = 3 * GAP + K    # 114 partitions holding the replicated operands

F32 = mybir.dt.float32
F16 = mybir.dt.bfloat16  # fp16 ACT output crashed the exec unit on HW; bf16 is native
AF = mybir.ActivationFunctionType


def build_program(rows=ROWS, m_cols=M, repeats=1):
    """Build the per-core Bass program. rows/m_cols shrinkable for sim."""
    rb = rows // P
    nq = max(1, m_cols // QUAD)

    nc = bacc.Bacc()
    l = nc.declare_dram_parameter("l", [AUG, rows], F32, isOutput=False)
    r = nc.declare_dram_parameter("r", [AUG, m_cols], F32, isOutput=False)
    b = nc.declare_dram_parameter("b", [P, 1], F32, isOutput=False)
    out = nc.declare_dram_parameter("out", [rows, m_cols], F16, isOutput=True)

    with tile.TileContext(nc) as tc:
        with (
            tc.tile_pool(name="singles", bufs=1) as singles,
            tc.tile_pool(name="psum_pool", bufs=2, space="PSUM") as psum_pool,
            tc.tile_pool(name="obuf_pool", bufs=4) as obuf_pool,
        ):
            bt = singles.tile([P, 1], F32)
            nc.sync.dma_start(bt[:], b[:])
            Lt = singles.tile([AUG, rows], F32)
            nc.sync.dma_start(Lt[:], l[:])
            Rt = singles.tile([AUG, m_cols], F32)
            # Load R's first 2048 columns separately so the first matmuls
            # only wait on ~1 MB of the 3.7 MB operand.
            c0 = min(QUAD, m_cols)
            nc.sync.dma_start(Rt[:, 0:c0], r[:, 0:c0])
            if m_cols > c0:
                nc.sync.dma_start(Rt[:, c0:], r[:, c0:])

            for _rep in range(repeats):
                for m in range(rb):
                    for h in range(m_cols // HALF if m_cols >= HALF else 1):
                        hw_ = min(HALF, m_cols)
                        ob = obuf_pool.tile([P, hw_], F16, tag="ob", name="ob")
                        for qq in range(hw_ // QUAD if hw_ >= QUAD else 1):
                            q = h * (HALF // QUAD) + qq
                            qw = min(QUAD, hw_)
                            ps = psum_pool.tile([P, qw], F32, tag="ps", name="ps")
                            for k in range(qw // FREE):
                                n = q * (QUAD // FREE) + k
                                nc.tensor.matmul(
                                    ps[:, k * FREE : (k + 1) * FREE],
                                    Lt[GAP * k : GAP * k + K, m * P : (m + 1) * P],
                                    Rt[GAP * k : GAP * k + K,
                                       n * FREE : (n + 1) * FREE],
                                    start=True, stop=True,
                                    tile_position=(GAP * k, 0),
                                )
                            nc.scalar.activation(
                                ob[:, qq * QUAD : qq * QUAD + qw], ps[:],
                                AF.Exp, bias=bt[:], scale=-0.5,
                            )
                        nc.sync.dma_start(
                            out[m * P : (m + 1) * P, h * HALF : h * HALF + hw_],
                            ob[:],
                        )

    nc.compile()  # bacc lowering: splits multi-waits, reg alloc, etc.
    return nc


_PROGRAM_CACHE = {}


def get_program(rows=ROWS, m_cols=M, repeats=1):
    key = (rows, m_cols, repeats)
    if key not in _PROGRAM_CACHE:
        _PROGRAM_CACHE[key] = build_program(rows, m_cols, repeats)
    return _PROGRAM_CACHE[key]


def make_in_maps(U, V, weights, sn):
    U = np.asarray(U, dtype=np.float32)
    V = np.asarray(V, dtype=np.float32)
    w = np.asarray(weights, dtype=np.float32).reshape(D)
    snf = float(np.asarray(sn, dtype=np.float32).reshape(()))

    s = np.exp(-w.astype(np.float64))
    Us = U.astype(np.float64) * s
    Vs = V.astype(np.float64) * s
    u2 = np.sum(Us * Us, axis=1)                     # (N,)
    v2 = np.sum(Vs * Vs, axis=1)                     # (M,)

    r_small = np.empty((K, M), dtype=np.float32)
    r_small[0:D] = Vs.T
    r_small[D] = 1.0
    r_small[D + 1] = v2
    r_full = np.zeros((AUG, M), dtype=np.float32)
    for g in range(4):
        r_full[GAP * g : GAP * g + K] = r_small
    r_full = np.ascontiguousarray(r_full)

    bias = np.full((P, 1), 2.0 * snf, dtype=np.float32)

    in_maps = []
    for c in range(N_CORES):
        rs = slice(c * ROWS, (c + 1) * ROWS)
        l_small = np.empty((K, ROWS), dtype=np.float32)
        l_small[0:D] = -2.0 * Us[rs].T
        l_small[D] = u2[rs]
        l_small[D + 1] = 1.0
        l_full = np.zeros((AUG, ROWS), dtype=np.float32)
        for g in range(4):
            l_full[GAP * g : GAP * g + K] = l_small
        in_maps.append({
            "l": np.ascontiguousarray(l_full),
            "r": r_full,
            "b": bias,
        })
    return in_maps


def kernel(U, V, weights, sn):
    from concourse.bass_utils import run_bass_kernel_spmd

    nc = get_program()
    in_maps = make_in_maps(U, V, weights, sn)
    res = run_bass_kernel_spmd(nc, in_maps, core_ids=list(range(N_CORES)))
    return np.concatenate(
        [np.asarray(r["out"]).astype(np.float32) for r in res.results], axis=0
    )


# revision 7
# speedup vs baseline: 1.0475x; 1.0475x over previous
"""ARD-RBF covariance kernel for Trainium2 (Bass/Tile), 8-core row-parallel.

Math (matches the reference):
    s  = exp(-weights[:, 0])                      # (D,) inverse lengthscales
    Us = U * s ; Vs = V * s
    sq[i, j] = ||Us_i||^2 + ||Vs_j||^2 - 2 Us_i . Vs_j
    K[i, j]  = exp(2*sn) * exp(-0.5 * max(sq, 0))

Device strategy (per core, rows sharded 8 ways):
    The augmented operands are built on the HOST (U/V are only 8192x16, so
    prep is trivial numpy) and DMA'd in ready-to-use:
      L (114 x rows): 4 replicas, at partitions 0/32/64/96, of
          [-2*Us^T block ; ||Us||^2 row ; ones row]   (K = 18)
      R (114 x m_cols): 4 replicas of [Vs^T ; ones row ; ||Vs||^2 row]
    One augmented matmul then computes sq directly in PSUM; the 4 replicas
    let 4 matmuls run concurrently in the PE's four 32-row groups
    (tile_position row tiling), hiding fp32 LDWEIGHTS + 2-pass cost.
    A single ScalarE activation per [128, 2048] PSUM tile computes
      out = Exp(-0.5 * psum + 2*sn)   (PSUM -> SBUF, bf16 output)
    and the bf16 tiles are DMA'd out in 1 MB chunks; the host upcasts to
    fp32 (bf16 rounding is ~2e-3 relative, inside the 2e-2 gate).

Per core the bf16 output is 16 MB (~45 us at 358 GB/s HBM write); the
8.4M-element Exp on ScalarE (~1 elem/cycle/lane @ 1.2 GHz => ~60 us) is
the steady-state bottleneck, with PE and DMA hidden under it.
"""

import numpy as np

import concourse.bacc as bacc
import concourse.bass as bass  # noqa: F401  (AP helpers)
import concourse.mybir as mybir
import concourse.tile as tile

N, M, D = 8192, 8192, 16
N_CORES = 8
ROWS = N // N_CORES  # 1024 rows of U per core
P = 128              # output partitions per row block
FREE = 512           # matmul moving free dim (fp32 max)
QUAD = 2048          # one ACT instruction: 4 PSUM banks of f32
HALF = 4096          # one output store: [128, 4096] fp16 = 1 MB
K = D + 2            # augmented contraction dim
GAP = 32             # partition stride between the 4 operand replicas
AUG = 3 * GAP + K    # 114 partitions holding the replicated operands

F32 = mybir.dt.float32
F16 = mybir.dt.bfloat16  # fp16 ACT output crashed the exec unit on HW; bf16 is native
AF = mybir.ActivationFunctionType


def build_program(rows=ROWS, m_cols=M, repeats=1):
    """Build the per-core Bass program. rows/m_cols shrinkable for sim."""
    rb = rows // P

    nc = bacc.Bacc()
    l = nc.declare_dram_parameter("l", [AUG, rows], F32, isOutput=False)
    r = nc.declare_dram_parameter("r", [AUG, m_cols], F32, isOutput=False)
    b = nc.declare_dram_parameter("b", [P, 1], F32, isOutput=False)
    out = nc.declare_dram_parameter("out", [rows, m_cols], F16, isOutput=True)

    with tile.TileContext(nc) as tc:
        with (
            tc.tile_pool(name="singles", bufs=1) as singles,
            tc.tile_pool(name="psum_pool", bufs=2, space="PSUM") as psum_pool,
            tc.tile_pool(name="obuf_pool", bufs=4) as obuf_pool,
        ):
            bt = singles.tile([P, 1], F32)
            nc.sync.dma_start(bt[:], b[:])
            # Dummy activation: hoists the ~2.7us Exp table-set load off the
            # critical path (it overlaps the L/R input DMAs instead of
            # stalling the first real ACT).
            warm = singles.tile([P, 1], F32)
            nc.scalar.activation(warm[:], bt[:], AF.Exp)
            Lt = singles.tile([AUG, rows], F32)
            nc.sync.dma_start(Lt[:], l[:])
            Rt = singles.tile([AUG, m_cols], F32)
            # Chunk R's load so the first matmul waits on only 512 columns
            # (~230 KB) of the 3.7 MB operand.
            c0 = min(FREE, m_cols)
            c1 = min(QUAD, m_cols)
            nc.sync.dma_start(Rt[:, 0:c0], r[:, 0:c0])
            if c1 > c0:
                nc.sync.dma_start(Rt[:, c0:c1], r[:, c0:c1])
            if m_cols > c1:
                nc.sync.dma_start(Rt[:, c1:], r[:, c1:])

            for _rep in range(repeats):
                for m in range(rb):
                    for h in range(m_cols // HALF if m_cols >= HALF else 1):
                        hw_ = min(HALF, m_cols)
                        ob = obuf_pool.tile([P, hw_], F16, tag="ob", name="ob")
                        for qq in range(hw_ // QUAD if hw_ >= QUAD else 1):
                            q = h * (HALF // QUAD) + qq
                            qw = min(QUAD, hw_)
                            ps = psum_pool.tile([P, qw], F32, tag="ps", name="ps")
                            for k in range(qw // FREE):
                                n = q * (QUAD // FREE) + k
                                nc.tensor.matmul(
                                    ps[:, k * FREE : (k + 1) * FREE],
                                    Lt[GAP * k : GAP * k + K, m * P : (m + 1) * P],
                                    Rt[GAP * k : GAP * k + K,
                                       n * FREE : (n + 1) * FREE],
                                    start=True, stop=True,
                                    tile_position=(GAP * k, 0),
                                )
                            nc.scalar.activation(
                                ob[:, qq * QUAD : qq * QUAD + qw], ps[:],
                                AF.Exp, bias=bt[:], scale=-0.5,
                            )
                        nc.sync.dma_start(
                            out[m * P : (m + 1) * P, h * HALF : h * HALF + hw_],
                            ob[:],
                        )

    nc.compile()  # bacc lowering: splits multi-waits, reg alloc, etc.
    return nc


_PROGRAM_CACHE = {}


def get_program(rows=ROWS, m_cols=M, repeats=1):
    key = (rows, m_cols, repeats)
    if key not in _PROGRAM_CACHE:
        _PROGRAM_CACHE[key] = build_program(rows, m_cols, repeats)
    return _PROGRAM_CACHE[key]


def make_in_maps(U, V, weights, sn):
    U = np.asarray(U, dtype=np.float32)
    V = np.asarray(V, dtype=np.float32)
    w = np.asarray(weights, dtype=np.float32).reshape(D)
    snf = float(np.asarray(sn, dtype=np.float32).reshape(()))

    s = np.exp(-w.astype(np.float64))
    Us = U.astype(np.float64) * s
    Vs = V.astype(np.float64) * s
    u2 = np.sum(Us * Us, axis=1)                     # (N,)
    v2 = np.sum(Vs * Vs, axis=1)                     # (M,)

    r_small = np.empty((K, M), dtype=np.float32)
    r_small[0:D] = Vs.T
    r_small[D] = 1.0
    r_small[D + 1] = v2
    r_full = np.zeros((AUG, M), dtype=np.float32)
    for g in range(4):
        r_full[GAP * g : GAP * g + K] = r_small
    r_full = np.ascontiguousarray(r_full)

    bias = np.full((P, 1), 2.0 * snf, dtype=np.float32)

    in_maps = []
    for c in range(N_CORES):
        rs = slice(c * ROWS, (c + 1) * ROWS)
        l_small = np.empty((K, ROWS), dtype=np.float32)
        l_small[0:D] = -2.0 * Us[rs].T
        l_small[D] = u2[rs]
        l_small[D + 1] = 1.0
        l_full = np.zeros((AUG, ROWS), dtype=np.float32)
        for g in range(4):
            l_full[GAP * g : GAP * g + K] = l_small
        in_maps.append({
            "l": np.ascontiguousarray(l_full),
            "r": r_full,
            "b": bias,
        })
    return in_maps


def kernel(U, V, weights, sn):
    from concourse.bass_utils import run_bass_kernel_spmd

    nc = get_program()
    in_maps = make_in_maps(U, V, weights, sn)
    res = run_bass_kernel_spmd(nc, in_maps, core_ids=list(range(N_CORES)))
    return np.concatenate(
        [np.asarray(r["out"]).astype(np.float32) for r in res.results], axis=0
    )
